# revision 20
# baseline (speedup 1.0000x reference)
"""GGNN (gated graph NN) message-passing kernel for 8 Trainium2 NeuronCores.

Sharding: edge-type sharding. Core c owns edge-type block c of the adjacency
matrix (columns c*N..(c+1)*N of the [N, 2E*N] adjacency, pre-transposed on the
host) plus node shard c (nodes 256c..256c+255) for the GRU update.

Per step, on core c:
  U-pre:  pz/pr += Uz/Ur-terms on local h shard   (covers the AllGather wait)
  stage1: t_c = h @ W_c                            [N, D]   fp16 out
  stage2: aT partials = (t_c)^T-contract A_cT      [D, N]   fp16 lhsT x fp16 rhs
          emitted in transposed orientation: out tile [f-chunk, dest-nodes]
  RS:     aT_shard = ReduceScatter_add(aT, fp16)   [D, N/8] in 2 f-halves
  GRU:    h_shard' = GRU(aT_shard, h_shard)        (fp32r matmuls)
  AG:     h^T' = AllGather(h_shard'^T, fp16)

Numerics: stage1+GRU matmuls float32r (12-bit mantissa, full PE rate at
free-dim>=256); stage2 fp16 x fp16 (adjacency 0/1 exact in fp16); RS/AG wires
fp16; fp32 accumulation in PSUM; elementwise GRU update fp32.
CPU-model predicted rel err ~9.6e-3 (vs 2e-2 gate).
"""
import sys
if "/opt/trn_rl_repo" not in sys.path:
    sys.path.insert(0, "/opt/trn_rl_repo")

import numpy as np

NC_CORES = 8
N = 2048          # nodes
D = 512           # state dim
ANN = 256         # annotation dim
STEPS = 5
SH = N // NC_CORES   # 256 nodes per shard
KT = D // 128        # 4
MT = N // 128        # 16


def _q12(x):
    """Round fp32 to 12 explicit mantissa bits (fp32r grid), RNE."""
    mant, ex = np.frexp(np.asarray(x, np.float32).astype(np.float64))
    return (np.round(mant * 4096) / 4096 * np.exp2(ex)).astype(np.float32)


DEBUG_DUMPS = False


def build(repeats=1, ablate=()):
    import concourse.bacc as bacc
    import concourse.mybir as mybir
    import concourse.tile as tile

    dt = mybir.dt
    nc = bacc.Bacc()
    if DEBUG_DUMPS:
        DBG_S = 1
        dbg_t_p = nc.declare_dram_parameter("dbg_t", [N, D], dt.float16,
                                            isOutput=True)
        dbg_rs_p = nc.declare_dram_parameter("dbg_rs", [D, SH], dt.float16,
                                             isOutput=True)
        dbg_z_p = nc.declare_dram_parameter("dbg_z", [D, SH], dt.float32,
                                            isOutput=True)
        dbg_a32_p = nc.declare_dram_parameter("dbg_a32", [D, SH], dt.float32r,
                                              isOutput=True)
        dbg_hp_p = nc.declare_dram_parameter("dbg_hp", [D, SH], dt.float32,
                                             isOutput=True)
        dbg_ag_p = nc.declare_dram_parameter("dbg_ag", [NC_CORES * D, SH],
                                             dt.float16, isOutput=True)
    at_p = nc.declare_dram_parameter("at", [N, N], dt.float16, isOutput=False)
    h0ag_p = nc.declare_dram_parameter("h0ag", [NC_CORES * D, SH], dt.float16,
                                       isOutput=False)
    h0sr_p = nc.declare_dram_parameter("h0sr", [D, SH], dt.float32r, isOutput=False)
    h0s_p = nc.declare_dram_parameter("h0s", [D, SH], dt.float32, isOutput=False)
    wc_p = nc.declare_dram_parameter("wc", [D, D], dt.float32r, isOutput=False)
    gw_p = nc.declare_dram_parameter("gw", [6, D, D], dt.float32r, isOutput=False)
    bpc_p = nc.declare_dram_parameter("bpc", [1, D], dt.float32, isOutput=False)
    bz_p = nc.declare_dram_parameter("bzc", [D, 1], dt.float32, isOutput=False)
    br_p = nc.declare_dram_parameter("brc", [D, 1], dt.float32, isOutput=False)
    bh_p = nc.declare_dram_parameter("bhc", [D, 1], dt.float32, isOutput=False)
    out_p = nc.declare_dram_parameter("out", [D, SH], dt.float32, isOutput=True)
    RG = [list(range(NC_CORES))]
    Act = mybir.ActivationFunctionType

    from contextlib import ExitStack
    with tile.TileContext(nc) as tc, ExitStack() as stk:
        res = stk.enter_context(tc.tile_pool(name="res", bufs=1))
        # PSUM: p_u holds the 4 z/r gate accumulators for the whole step;
        # p_acc (4 banks) cycles through stage1 pt / stage2 paT / ht.
        p_u = stk.enter_context(tc.tile_pool(name="pu", bufs=1, space="PSUM"))
        p_acc = stk.enter_context(tc.tile_pool(name="pacc", bufs=4, space="PSUM"))
        p_hc = stk.enter_context(tc.tile_pool(name="phc", bufs=3))
        p_hc32 = stk.enter_context(tc.tile_pool(name="phc32", bufs=3))
        p_t = stk.enter_context(tc.tile_pool(name="pt", bufs=1))
        p_asb = stk.enter_context(tc.tile_pool(name="pasb", bufs=4))
        p_an = stk.enter_context(tc.tile_pool(name="pan", bufs=2))
        p_sm = stk.enter_context(tc.tile_pool(name="psm", bufs=1))
        p_tmp = stk.enter_context(tc.tile_pool(name="ptmp", bufs=3))
        p_h = stk.enter_context(tc.tile_pool(name="ph", bufs=2))
        dram = stk.enter_context(tc.tile_pool(name="dram", bufs=2, space="DRAM"))

        # ---- setup: constants, weights, adjacency ----
        ones = res.tile([1, 128], dt.float32, tag="ones")
        nc.vector.memset(ones[:], 1.0)
        bpc_t = res.tile([1, D], dt.float32, tag="bpc")
        nc.sync.dma_start(bpc_t[:], bpc_p[:])
        pb = p_acc.tile([128, D], dt.float32, tag="mm")
        nc.tensor.matmul(pb[:], ones[:], bpc_t[:], start=True, stop=True)
        bias_bcast = res.tile([128, D], dt.float32, tag="bias_bcast")
        nc.vector.tensor_copy(bias_bcast[:], pb[:])

        bias_tiles = {}
        for nm, par in (("z", bz_p), ("r", br_p), ("h", bh_p)):
            for f in range(KT):
                bt = res.tile([128, 1], dt.float32, tag=f"b{nm}{f}")
                nc.sync.dma_start(bt[:], par[f * 128:(f + 1) * 128, :])
                bias_tiles[(nm, f)] = bt

        wc_t = []
        for k in range(KT):
            w = res.tile([128, D], dt.float32r, tag=f"wc{k}")
            nc.sync.dma_start(w[:], wc_p[k * 128:(k + 1) * 128, :])
            wc_t.append(w)

        at_t = []
        for m in range(MT):
            a = res.tile([128, N], dt.float16, tag=f"at{m}")
            nc.sync.dma_start(a[:], at_p[m * 128:(m + 1) * 128, :])
            at_t.append(a)

        gw_res = []
        for g in range(6):
            w = res.tile([128, KT, D], dt.float32r, tag=f"gwr{g}")
            nc.scalar.dma_start(w[:], gw_p[g].rearrange("(k p) f -> p k f", p=128))
            gw_res.append(w)

        for rep in range(repeats):
          hsh_prev = []   # h^T shard, fp32r (GRU rhs)
          h32_prev = []   # h^T shard, fp32 (elementwise state)
          for k in range(KT):
            hr = p_h.tile([128, SH], dt.float32r, tag=f"hnr{k}")
            nc.sync.dma_start(hr[:], h0sr_p[k * 128:(k + 1) * 128, :])
            hsh_prev.append(hr)
            h3 = p_h.tile([128, SH], dt.float32, tag=f"h32{k}")
            nc.sync.dma_start(h3[:], h0s_p[k * 128:(k + 1) * 128, :])
            h32_prev.append(h3)

          ag_out_prev = None

          for s in range(STEPS):
             # ---- z/r U-terms (local h only; cover the AG wait) ----
             # zr_ps[g][fp]: [128, 512] psum, f-chunk (2*fp+j) at cols j*SH
             zr_ps = {}
             for g, uidx in (("z", 1), ("r", 3)):
                 for fp in range(2):
                     zr_ps[(g, fp)] = p_u.tile([128, 2 * SH], dt.float32,
                                               tag=f"u{g}{fp}",
                                               name=f"u{g}{fp}")
             if "gru" not in ablate:
                 # one accumulation group per PSUM bank: start only on the
                 # very first matmul touching the bank (j==0, k==0); the
                 # matching stop is on the last W-term matmul below.
                 for g, uidx in (("z", 1), ("r", 3)):
                     Uq = gw_res[uidx]
                     for fp in range(2):
                         for j in range(2):
                             f = 2 * fp + j
                             for k in range(KT):
                                 nc.tensor.matmul(
                                     zr_ps[(g, fp)][:, j * SH:(j + 1) * SH],
                                     Uq[:, k, f * 128:(f + 1) * 128],
                                     hsh_prev[k][:],
                                     start=(j == 0 and k == 0), stop=False)

             # ---- stage 1: t = h @ W_c (+ b_c); t stored fp16 ----
             t_tiles = [None] * MT
             for m in range(MT):
                 pt = p_acc.tile([128, D], dt.float32, tag="mm")
                 if "s1" in ablate:
                     nc.tensor.matmul(pt[:], wc_t[0][:, 0:128], wc_t[1][:],
                                      start=True, stop=True)
                 else:
                     hc16 = p_hc.tile([128, KT, 128], dt.float16, tag="hc")
                     src = (h0ag_p if s == 0 else ag_out_prev)
                     c_blk = src[(m // 2) * D:(m // 2 + 1) * D,
                                 (m % 2) * 128:(m % 2) * 128 + 128]
                     nc.sync.dma_start(
                         hc16[:], c_blk.rearrange("(k p) n -> p k n", p=128))
                     hc32 = p_hc32.tile([128, KT, 128], dt.float32r, tag="hc32")
                     nc.scalar.copy(hc32[:], hc16[:])
                     for k in range(KT):
                         nc.tensor.matmul(pt[:], hc32[:, k, :], wc_t[k][:],
                                          start=(k == 0), stop=(k == KT - 1))
                 tm = p_t.tile([128, D], dt.float16, tag=f"t{m}")
                 nc.vector.tensor_add(tm[:], pt[:], bias_bcast[:])
                 t_tiles[m] = tm
                 if DEBUG_DUMPS and s == DBG_S and rep == 0:
                     nc.scalar.dma_start(dbg_t_p[m * 128:(m + 1) * 128, :], tm[:])

             # ---- stage 2: aT partials, fp16 x fp16, transposed output ----
             # out tile [f-chunk 128, dest 512]; k-contract over src tiles.
             # rs_in[h] rows sc*256 + fr  <->  aT[h*256+fr, shard sc]
             rs_ins = []
             rs_outs = []
             for h in range(2):
                 rs_ins.append(dram.tile([N, SH], dt.float16, tag=f"rs_in{h}",
                                         name=f"rs_in{h}"))
             asb_cnt = 0
             for fc in range(KT):
                 h = fc // 2
                 for dhalf in range(2):
                     pas = [p_acc.tile([128, D], dt.float32, tag="mm",
                                       name=f"pa{fc}_{dhalf}_{j}")
                            for j in range(2)]
                     if "s2" in ablate:
                         for j in range(2):
                             nc.tensor.matmul(pas[j][:], t_tiles[0][:, 0:128],
                                              t_tiles[1][:], start=True, stop=True)
                     else:
                         for k in range(MT):
                             lhs = t_tiles[k][:, fc * 128:(fc + 1) * 128]
                             for j in range(2):
                                 nc.tensor.matmul(
                                     pas[j][:], lhs,
                                     at_t[k][:, dhalf * 1024 + j * 512:
                                             dhalf * 1024 + (j + 1) * 512],
                                     start=(k == 0), stop=(k == MT - 1))
                     for j in range(2):
                         asb = p_asb.tile([128, D], dt.float16, tag="asb")
                         if asb_cnt % 2 == 0:
                             nc.scalar.copy(asb[:], pas[j][:])
                         else:
                             nc.vector.tensor_copy(asb[:], pas[j][:])
                         # dest rows: shards {sp, sp+1}, f-offset (fc%2)*128
                         sp = (dhalf * 2 + j) * 2
                         dst = rs_ins[h].rearrange("(s r) n2 -> r s n2", s=NC_CORES)
                         eng = nc.sync if asb_cnt % 2 == 0 else nc.scalar
                         eng.dma_start(
                             dst[(fc % 2) * 128:(fc % 2) * 128 + 128,
                                 sp:sp + 2, :],
                             asb[:].rearrange("p (s n2) -> p s n2", s=2))
                         asb_cnt += 1
                 if fc % 2 == 1:
                     rs_out = dram.tile([SH, SH], dt.float16, tag=f"rs_out{h}",
                                        name=f"rs_out{h}")
                     if "cc" in ablate or "rs" in ablate:
                         nc.sync.dma_start(rs_out[:], rs_ins[h][0:SH, :])
                     else:
                         nc.gpsimd.collective_compute(
                             "ReduceScatter", mybir.AluOpType.add,
                             replica_groups=RG,
                             ins=[rs_ins[h][:]], outs=[rs_out[:]])
                     rs_outs.append(rs_out)

             # ---- aT shard upconvert fp16 -> fp32r ----
             aT = [None] * KT
             for h in range(2):
                 an16 = p_an.tile([128, 2, SH], dt.float16, tag="an16")
                 nc.sync.dma_start(
                     an16[:], rs_outs[h][:].rearrange("(j p) n -> p j n", p=128))
                 for j in range(2):
                     k = 2 * h + j
                     a_k = p_sm.tile([128, SH], dt.float32r, tag=f"aT{k}")
                     nc.vector.tensor_copy(a_k[:], an16[:, j, :])
                     aT[k] = a_k
                     if DEBUG_DUMPS and s == DBG_S and rep == 0:
                         nc.scalar.dma_start(
                             dbg_rs_p[k * 128:(k + 1) * 128, :], an16[:, j, :])
                         nc.scalar.dma_start(
                             dbg_a32_p[k * 128:(k + 1) * 128, :], a_k[:])

             # ---- GRU gates ----
             if "gru" in ablate:
                 for g, widx in (("z", 0), ("r", 2)):
                     for fp in range(2):
                         for j in range(2):
                             nc.tensor.matmul(
                                 zr_ps[(g, fp)][:, j * SH:(j + 1) * SH],
                                 aT[0][:, 0:128], aT[0][:],
                                 start=(j == 0), stop=(j == 1))
             else:
                 # W-terms continue the U accumulation; k-major so k={0,1}
                 # (from RS half 0) can start while RS half 1 is in flight.
                 for k in range(KT):
                     for g, widx in (("z", 0), ("r", 2)):
                         Wq = gw_res[widx]
                         for fp in range(2):
                             for j in range(2):
                                 f = 2 * fp + j
                                 nc.tensor.matmul(
                                     zr_ps[(g, fp)][:, j * SH:(j + 1) * SH],
                                     Wq[:, k, f * 128:(f + 1) * 128],
                                     aT[k][:],
                                     start=False,
                                     stop=(k == KT - 1 and j == 1))
             z_t, r_t = [], []
             for g, outs in (("z", z_t), ("r", r_t)):
                 for f in range(KT):
                     og = p_sm.tile([128, SH], dt.float32, tag=f"g{g}{f}",
                                    name=f"g{g}{f}")
                     nc.scalar.activation(
                         og[:], zr_ps[(g, f // 2)][:, (f % 2) * SH:
                                                   (f % 2 + 1) * SH],
                         Act.Sigmoid, bias=bias_tiles[(g, f)][:])
                     outs.append(og)
                     if DEBUG_DUMPS and s == DBG_S and rep == 0 and g == "z":
                         nc.scalar.dma_start(
                             dbg_z_p[f * 128:(f + 1) * 128, :], og[:])
             rh = []
             for k in range(KT):
                 rhk = p_sm.tile([128, SH], dt.float32r, tag=f"rh{k}")
                 nc.vector.tensor_mul(rhk[:], r_t[k][:], h32_prev[k][:])
                 rh.append(rhk)
             # h-tilde gate: U-term on rh, then W-term on aT
             ht_ps = [p_acc.tile([128, 2 * SH], dt.float32, tag="mm",
                                 name=f"ht{fp}") for fp in range(2)]
             if "gru" in ablate:
                 for fp in range(2):
                     nc.tensor.matmul(ht_ps[fp][:, 0:SH], rh[0][:, 0:128],
                                      rh[0][:], start=True, stop=True)
                     nc.tensor.matmul(ht_ps[fp][:, SH:2 * SH], rh[0][:, 0:128],
                                      rh[0][:], start=True, stop=True)
             else:
                 for fp in range(2):
                     for j in range(2):
                         f = 2 * fp + j
                         for k in range(KT):
                             nc.tensor.matmul(
                                 ht_ps[fp][:, j * SH:(j + 1) * SH],
                                 gw_res[5][:, k, f * 128:(f + 1) * 128],
                                 rh[k][:],
                                 start=(j == 0 and k == 0), stop=False)
                         for k in range(KT):
                             nc.tensor.matmul(
                                 ht_ps[fp][:, j * SH:(j + 1) * SH],
                                 gw_res[4][:, k, f * 128:(f + 1) * 128],
                                 aT[k][:], start=False,
                                 stop=(k == KT - 1 and j == 1))
             ht_t = []
             for f in range(KT):
                 og = p_sm.tile([128, SH], dt.float32, tag=f"gh{f}",
                                name=f"gh{f}")
                 nc.scalar.activation(
                     og[:], ht_ps[f // 2][:, (f % 2) * SH:(f % 2 + 1) * SH],
                     Act.Tanh, bias=bias_tiles[("h", f)][:])
                 ht_t.append(og)

             # ---- h' = h + z * (ht - h); AG in fp16 ----
             last = (s == STEPS - 1)
             if not last:
                 ag_in = dram.tile([D, SH], dt.float16, tag="ag_in",
                                   name="ag_in")
             hsh_new, h32_new = [], []
             for k in range(KT):
                 s1 = p_tmp.tile([128, SH], dt.float32, tag="gsA")
                 nc.vector.tensor_sub(s1[:], ht_t[k][:], h32_prev[k][:])
                 s2 = p_tmp.tile([128, SH], dt.float32, tag="gsB")
                 nc.vector.tensor_mul(s2[:], z_t[k][:], s1[:])
                 h3 = p_h.tile([128, SH], dt.float32, tag=f"h32{k}")
                 nc.vector.tensor_add(h3[:], h32_prev[k][:], s2[:])
                 h32_new.append(h3)
                 if DEBUG_DUMPS and s == DBG_S - 1 and rep == 0:
                     nc.scalar.dma_start(dbg_hp_p[k * 128:(k + 1) * 128, :],
                                         h3[:])
                 if last:
                     nc.sync.dma_start(out_p[k * 128:(k + 1) * 128, :], h3[:])
                 else:
                     hr = p_h.tile([128, SH], dt.float32r, tag=f"hnr{k}")
                     nc.vector.tensor_copy(hr[:], h3[:])
                     hsh_new.append(hr)
                     h16 = p_tmp.tile([128, SH], dt.float16, tag="h16")
                     nc.scalar.copy(h16[:], h3[:])
                     nc.sync.dma_start(ag_in[k * 128:(k + 1) * 128, :], h16[:])

             if not last:
                 ag_out = dram.tile([NC_CORES * D, SH], dt.float16,
                                    tag="ag_out", name="ag_out",
                                    addr_space="Shared")
                 if "cc" in ablate or "ag" in ablate:
                     nc.sync.dma_start(ag_out[0:D, :], ag_in[:])
                 else:
                     nc.gpsimd.collective_compute(
                         "AllGather", mybir.AluOpType.bypass, replica_groups=RG,
                         ins=[ag_in[:]], outs=[ag_out[:]])
                 if DEBUG_DUMPS and s == DBG_S - 1 and rep == 0:
                     nc.sync.dma_start(dbg_ag_p[:], ag_out[:])
                 ag_out_prev = ag_out
                 hsh_prev, h32_prev = hsh_new, h32_new

    nc.finalize()
    return nc


_BUILT = None
TRACE = False
LAST_RESULT = None
_BUILT_R = {}


def _get_built(repeats=1, ablate=()):
    global _BUILT
    key = (repeats, tuple(ablate))
    if key != (1, ()):
        if key not in _BUILT_R:
            _BUILT_R[key] = build(repeats, ablate)
        return _BUILT_R[key]
    if _BUILT is None:
        _BUILT = build()
    return _BUILT


def prepare_in_maps(adjacency, annotations, W_prop, b_prop, Wz, Uz, bz,
                    Wr, Ur, br, Wh, Uh, bh):
    import ml_dtypes  # noqa: F401  (float16 is numpy-native; kept for parity)
    A = np.asarray(adjacency, np.float32)
    ann = np.asarray(annotations, np.float32)
    W_prop = np.asarray(W_prop, np.float32)
    b_prop = np.asarray(b_prop, np.float32)
    gw_all = _q12(np.stack([np.asarray(x, np.float32)
                            for x in (Wz, Uz, Wr, Ur, Wh, Uh)]))
    bz = np.asarray(bz, np.float32).reshape(D, 1)
    br = np.asarray(br, np.float32).reshape(D, 1)
    bh = np.asarray(bh, np.float32).reshape(D, 1)

    h0 = np.zeros((N, D), np.float32)
    h0[:, :ann.shape[1]] = ann
    h0t = np.ascontiguousarray(h0.T)           # [D, N] fp32
    h0t_r = _q12(h0t)
    A_T = np.ascontiguousarray(A.T)            # [2E*N, N]

    # natural shard order: core c owns nodes 256c..256c+255
    h0ag = np.ascontiguousarray(
        h0t.reshape(D, NC_CORES, SH).transpose(1, 0, 2).reshape(
            NC_CORES * D, SH)).astype(np.float16)

    in_maps = []
    for c in range(NC_CORES):
        in_maps.append({
            "at": np.ascontiguousarray(
                A_T[c * N:(c + 1) * N, :]).astype(np.float16),
            "h0ag": h0ag,
            "h0sr": np.ascontiguousarray(h0t_r[:, c * SH:(c + 1) * SH]),
            "h0s": np.ascontiguousarray(h0t[:, c * SH:(c + 1) * SH]),
            "wc": _q12(W_prop[c]),
            "gw": gw_all,
            "bpc": np.ascontiguousarray(b_prop[c].reshape(1, D)),
            "bzc": bz, "brc": br, "bhc": bh,
        })
    return in_maps


def kernel(**inputs):
    from concourse.bass_utils import run_bass_kernel_spmd

    in_maps = prepare_in_maps(
        **{k: inputs[k] for k in ("adjacency", "annotations", "W_prop", "b_prop",
                                  "Wz", "Uz", "bz", "Wr", "Ur", "br",
                                  "Wh", "Uh", "bh")})
    nc = _get_built()
    res = run_bass_kernel_spmd(nc, in_maps, list(range(NC_CORES)), trace=TRACE)
    global LAST_RESULT
    LAST_RESULT = res
    h = np.empty((N, D), np.float32)
    for c in range(NC_CORES):
        h[c * SH:(c + 1) * SH] = res.results[c]["out"].T
    return h


# revision 28
# speedup vs baseline: 1.0446x; 1.0446x over previous
"""GGNN (gated graph NN) message-passing kernel for 8 Trainium2 NeuronCores.

Sharding: edge-type sharding. Core c owns edge-type block c of the adjacency
matrix (columns c*N..(c+1)*N of the [N, 2E*N] adjacency, pre-transposed on the
host) plus node shard c (nodes 256c..256c+255) for the GRU update.

Per step, on core c:
  U-pre:  pz/pr += Uz/Ur-terms on local h shard   (covers the AllGather wait)
  stage1: t_c = h @ W_c                            [N, D]   fp16 out
          (step 0 contracts only d<256: h0 columns 256.. are zero)
  stage2: aT partials, transposed orientation      [D, N]   fp16 x fp16
  RS:     4x quarter ReduceScatter (fp16), one per f-chunk; each quarter's
          output IS the aT k-chunk shard (cast-DMA'd to fp32r)
  GRU:    h_shard' = GRU(aT_shard, h_shard)        (fp32r matmuls)
  AG:     h^T' = AllGather(h_shard'^T, fp16)

Numerics: stage1+GRU matmuls float32r (full PE rate at free-dim>=256);
stage2 fp16 x fp16 (adjacency 0/1 exact in fp16); RS/AG wires fp16; fp32
accumulation in PSUM; h state kept in fp32r; elementwise tail fused into
wide [128, 512] ops split across DVE and Pool.
"""
import sys
if "/opt/trn_rl_repo" not in sys.path:
    sys.path.insert(0, "/opt/trn_rl_repo")

import numpy as np

NC_CORES = 8
N = 2048          # nodes
D = 512           # state dim
ANN = 256         # annotation dim
STEPS = 5
SH = N // NC_CORES   # 256 nodes per shard
KT = D // 128        # 4
MT = N // 128        # 16


def _q12(x):
    """Round fp32 to 12 explicit mantissa bits (fp32r grid), RNE."""
    mant, ex = np.frexp(np.asarray(x, np.float32).astype(np.float64))
    return (np.round(mant * 4096) / 4096 * np.exp2(ex)).astype(np.float32)


DEBUG_DUMPS = False


def build(repeats=1, ablate=()):
    import concourse.bacc as bacc
    import concourse.mybir as mybir
    import concourse.tile as tile

    dt = mybir.dt
    nc = bacc.Bacc()
    at_p = nc.declare_dram_parameter("at", [N, N], dt.float16, isOutput=False)
    h0ag_p = nc.declare_dram_parameter("h0ag", [NC_CORES * D, SH], dt.float16,
                                       isOutput=False)
    h0sr_p = nc.declare_dram_parameter("h0sr", [D, SH], dt.float32r, isOutput=False)
    wc_p = nc.declare_dram_parameter("wc", [D, D], dt.float32r, isOutput=False)
    gw_p = nc.declare_dram_parameter("gw", [6, D, D], dt.float32r, isOutput=False)
    bpc_p = nc.declare_dram_parameter("bpc", [1, D], dt.float32, isOutput=False)
    bz_p = nc.declare_dram_parameter("bzc", [D, 1], dt.float32, isOutput=False)
    br_p = nc.declare_dram_parameter("brc", [D, 1], dt.float32, isOutput=False)
    bh_p = nc.declare_dram_parameter("bhc", [D, 1], dt.float32, isOutput=False)
    out_p = nc.declare_dram_parameter("out", [D, SH], dt.float32r, isOutput=True)
    if DEBUG_DUMPS:
        dbg_hp_p = nc.declare_dram_parameter("dbg_hp", [D, SH], dt.float32r,
                                             isOutput=True)
        dbg_a32_p = nc.declare_dram_parameter("dbg_a32", [D, SH], dt.float32r,
                                              isOutput=True)
    RG = [list(range(NC_CORES))]
    Act = mybir.ActivationFunctionType

    from contextlib import ExitStack
    with tile.TileContext(nc) as tc, ExitStack() as stk:
        res = stk.enter_context(tc.tile_pool(name="res", bufs=1))
        # PSUM: p_u holds the 4 z/r gate accumulators across the step;
        # p_acc (4 banks) cycles through stage1 pt / stage2 paT / ht.
        p_u = stk.enter_context(tc.tile_pool(name="pu", bufs=1, space="PSUM"))
        p_acc = stk.enter_context(tc.tile_pool(name="pacc", bufs=4, space="PSUM"))
        p_hc16 = stk.enter_context(tc.tile_pool(name="phc16", bufs=2))
        p_hc32 = stk.enter_context(tc.tile_pool(name="phc32", bufs=2))
        p_t = stk.enter_context(tc.tile_pool(name="pt", bufs=1))
        p_asb = stk.enter_context(tc.tile_pool(name="pasb", bufs=4))
        p_sm = stk.enter_context(tc.tile_pool(name="psm", bufs=1))
        p_tmp = stk.enter_context(tc.tile_pool(name="ptmp", bufs=1))
        p_h = stk.enter_context(tc.tile_pool(name="ph", bufs=2))
        dram = stk.enter_context(tc.tile_pool(name="dram", bufs=2, space="DRAM"))

        # ---- setup: constants, weights, adjacency ----
        ones = res.tile([1, 128], dt.float32, tag="ones")
        nc.vector.memset(ones[:], 1.0)
        bpc_t = res.tile([1, D], dt.float32, tag="bpc")
        nc.sync.dma_start(bpc_t[:], bpc_p[:])
        pb = p_acc.tile([128, D], dt.float32, tag="mm")
        nc.tensor.matmul(pb[:], ones[:], bpc_t[:], start=True, stop=True)
        bias_bcast = res.tile([128, D], dt.float32, tag="bias_bcast")
        nc.vector.tensor_copy(bias_bcast[:], pb[:])

        bias_tiles = {}
        for nm, par in (("z", bz_p), ("r", br_p), ("h", bh_p)):
            for f in range(KT):
                bt = res.tile([128, 1], dt.float32, tag=f"b{nm}{f}")
                nc.sync.dma_start(bt[:], par[f * 128:(f + 1) * 128, :])
                bias_tiles[(nm, f)] = bt

        wc_t = []
        for k in range(KT):
            w = res.tile([128, D], dt.float32r, tag=f"wc{k}")
            nc.sync.dma_start(w[:], wc_p[k * 128:(k + 1) * 128, :])
            wc_t.append(w)

        at_t = []
        for m in range(MT):
            a = res.tile([128, N], dt.float16, tag=f"at{m}")
            nc.sync.dma_start(a[:], at_p[m * 128:(m + 1) * 128, :])
            at_t.append(a)

        gw_res = []
        for g in range(6):
            w = res.tile([128, KT, D], dt.float32r, tag=f"gwr{g}")
            nc.scalar.dma_start(w[:], gw_p[g].rearrange("(k p) f -> p k f", p=128))
            gw_res.append(w)

        for rep in range(repeats):
          # h state: one fp32r big tile [128, KT, SH]; slice [:, k, :] is the
          # matmul rhs for k-chunk k.
          h32_prev = p_h.tile([128, KT, SH], dt.float32r, tag="h32",
                              name="h32_init")
          nc.sync.dma_start(h32_prev[:],
                            h0sr_p[:].rearrange("(k p) n -> p k n", p=128))

          ag_out_prev = None

          for s in range(STEPS):
             KT_s = 2 if s == 0 else KT   # h0 cols 256.. are zero
             # ---- z/r U-terms (local h only; cover the AG wait) ----
             zr_ps = {}
             for g in ("z", "r"):
                 for fp in range(2):
                     zr_ps[(g, fp)] = p_u.tile([128, 2 * SH], dt.float32,
                                               tag=f"u{g}{fp}",
                                               name=f"u{g}{fp}")
             if "gru" not in ablate:
                 for g, uidx in (("z", 1), ("r", 3)):
                     Uq = gw_res[uidx]
                     for fp in range(2):
                         for j in range(2):
                             f = 2 * fp + j
                             for k in range(KT):
                                 nc.tensor.matmul(
                                     zr_ps[(g, fp)][:, j * SH:(j + 1) * SH],
                                     Uq[:, k, f * 128:(f + 1) * 128],
                                     h32_prev[:, k, :],
                                     start=(j == 0 and k == 0), stop=False)

             # ---- stage 1: t = h @ W_c (+ b_c); t stored fp16 ----
             # hT arrives via 4 cast-DMAs (fp16 -> fp32r) of 2 shards each.
             hc32 = []
             if "s1" not in ablate:
                 src = (h0ag_p if s == 0 else ag_out_prev)
                 view = src[:].rearrange("(c k p) n -> p k c n", p=128, k=KT)
                 for cp in range(4):
                     hc16 = p_hc16.tile([128, KT, 2, SH], dt.float16,
                                        tag="hc16")
                     hc = p_hc32.tile([128, KT, 2, SH], dt.float32r, tag="hc32")
                     for ci in range(2):
                         nc.sync.dma_start(hc16[:, :, ci, :],
                                           view[:, :, 2 * cp + ci, :])
                         nc.scalar.copy(hc[:, :, ci, :], hc16[:, :, ci, :])
                     hc32.append(hc)
             t_tiles = [None] * MT
             for m in range(MT):
                 pt = p_acc.tile([128, D], dt.float32, tag="mm")
                 if "s1" in ablate:
                     nc.tensor.matmul(pt[:], wc_t[0][:, 0:128], wc_t[1][:],
                                      start=True, stop=True)
                 else:
                     hc = hc32[m // 4]
                     ci, half = (m // 2) % 2, m % 2
                     for k in range(KT_s):
                         nc.tensor.matmul(
                             pt[:], hc[:, k, ci, half * 128:half * 128 + 128],
                             wc_t[k][:], start=(k == 0), stop=(k == KT_s - 1))
                 tm = p_t.tile([128, D], dt.float16, tag=f"t{m}")
                 nc.vector.tensor_add(tm[:], pt[:], bias_bcast[:])
                 t_tiles[m] = tm

             # ---- stage 2 + quarter RS: aT in transposed orientation ----
             # quarter fc covers f-rows fc*128..fc*128+127; its RS output is
             # exactly aT k-chunk fc's shard.
             aT = [None] * KT
             for fc in range(KT):
                 rs_in = dram.tile([N // 2, SH], dt.float16, tag=f"rs_in{fc}",
                                   name=f"rs_in{fc}")
                 asb_cnt = 0
                 for dhalf in range(2):
                     pas = [p_acc.tile([128, D], dt.float32, tag="mm",
                                       name=f"pa{fc}_{dhalf}_{j}")
                            for j in range(2)]
                     if "s2" in ablate:
                         for j in range(2):
                             nc.tensor.matmul(pas[j][:], t_tiles[0][:, 0:128],
                                              t_tiles[1][:], start=True,
                                              stop=True)
                     else:
                         for k in range(MT):
                             lhs = t_tiles[k][:, fc * 128:(fc + 1) * 128]
                             for j in range(2):
                                 nc.tensor.matmul(
                                     pas[j][:], lhs,
                                     at_t[k][:, dhalf * 1024 + j * 512:
                                             dhalf * 1024 + (j + 1) * 512],
                                     start=(k == 0), stop=(k == MT - 1))
                     for j in range(2):
                         asb = p_asb.tile([128, D], dt.float16, tag="asb")
                         if asb_cnt % 2 == 0:
                             nc.scalar.copy(asb[:], pas[j][:])
                         else:
                             nc.vector.tensor_copy(asb[:], pas[j][:])
                         # dest rows: shard-pair sp covers shards 2sp, 2sp+1
                         sp = dhalf * 2 + j
                         dst = rs_in[:].rearrange(
                             "(q s2 p) n2 -> p q s2 n2", q=4, s2=2, p=128)
                         eng = nc.sync if asb_cnt % 2 == 0 else nc.scalar
                         eng.dma_start(
                             dst[:, sp, :, :],
                             asb[:].rearrange("p (s2 n2) -> p s2 n2", s2=2))
                         asb_cnt += 1
                 rs_out = dram.tile([SH // 2, SH], dt.float16,
                                    tag=f"rs_out{fc}", name=f"rs_out{fc}")
                 if "cc" in ablate or "rs" in ablate:
                     nc.sync.dma_start(rs_out[:], rs_in[0:SH // 2, :])
                 else:
                     nc.gpsimd.collective_compute(
                         "ReduceScatter", mybir.AluOpType.add,
                         replica_groups=RG,
                         ins=[rs_in[:]], outs=[rs_out[:]])
                 # rs_out [128, SH] fp16 == aT[fc] shard; cast-DMA to fp32r
                 a_k = p_sm.tile([128, SH], dt.float32r, tag=f"aT{fc}",
                                 name=f"aT{fc}")
                 nc.gpsimd.dma_start(a_k[:], rs_out[:])
                 aT[fc] = a_k
                 if DEBUG_DUMPS and s == 1 and rep == 0:
                     nc.scalar.dma_start(
                         dbg_a32_p[fc * 128:(fc + 1) * 128, :], a_k[:])

             # ---- GRU gates ----
             if "gru" in ablate:
                 for g in ("z", "r"):
                     for fp in range(2):
                         for j in range(2):
                             nc.tensor.matmul(
                                 zr_ps[(g, fp)][:, j * SH:(j + 1) * SH],
                                 aT[0][:, 0:128], aT[0][:],
                                 start=False, stop=(j == 1))
             else:
                 # W-terms k-major: k-chunk k runs as soon as quarter-RS k
                 # has landed (while later quarters are still in flight).
                 for k in range(KT):
                     for g, widx in (("z", 0), ("r", 2)):
                         Wq = gw_res[widx]
                         for fp in range(2):
                             for j in range(2):
                                 f = 2 * fp + j
                                 nc.tensor.matmul(
                                     zr_ps[(g, fp)][:, j * SH:(j + 1) * SH],
                                     Wq[:, k, f * 128:(f + 1) * 128],
                                     aT[k][:],
                                     start=False,
                                     stop=(k == KT - 1 and j == 1))
             z_big = p_sm.tile([128, KT, SH], dt.float32, tag="z_big")
             r_big = p_sm.tile([128, KT, SH], dt.float32, tag="r_big")
             for g, big in (("z", z_big), ("r", r_big)):
                 for f in range(KT):
                     nc.scalar.activation(
                         big[:, f, :], zr_ps[(g, f // 2)][:, (f % 2) * SH:
                                                          (f % 2 + 1) * SH],
                         Act.Sigmoid, bias=bias_tiles[(g, f)][:])
             rh_big = p_sm.tile([128, KT, SH], dt.float32r, tag="rh_big")
             nc.vector.tensor_mul(rh_big[:], r_big[:], h32_prev[:])
             # h-tilde gate: U-term on rh, then W-term on aT
             ht_ps = [p_acc.tile([128, 2 * SH], dt.float32, tag="mm",
                                 name=f"ht{fp}") for fp in range(2)]
             if "gru" in ablate:
                 for fp in range(2):
                     nc.tensor.matmul(ht_ps[fp][:, 0:SH], rh_big[:, 0, 0:128],
                                      rh_big[:, 0, :], start=True, stop=True)
                     nc.tensor.matmul(ht_ps[fp][:, SH:2 * SH],
                                      rh_big[:, 0, 0:128],
                                      rh_big[:, 0, :], start=True, stop=True)
             else:
                 for fp in range(2):
                     for j in range(2):
                         f = 2 * fp + j
                         for k in range(KT):
                             nc.tensor.matmul(
                                 ht_ps[fp][:, j * SH:(j + 1) * SH],
                                 gw_res[5][:, k, f * 128:(f + 1) * 128],
                                 rh_big[:, k, :],
                                 start=(j == 0 and k == 0), stop=False)
                         for k in range(KT):
                             nc.tensor.matmul(
                                 ht_ps[fp][:, j * SH:(j + 1) * SH],
                                 gw_res[4][:, k, f * 128:(f + 1) * 128],
                                 aT[k][:], start=False,
                                 stop=(k == KT - 1 and j == 1))
             ht_big = p_sm.tile([128, KT, SH], dt.float32, tag="ht_big")
             for f in range(KT):
                 nc.scalar.activation(
                     ht_big[:, f, :], ht_ps[f // 2][:, (f % 2) * SH:
                                                    (f % 2 + 1) * SH],
                     Act.Tanh, bias=bias_tiles[("h", f)][:])

             # ---- h' = h + z * (ht - h), wide ops split DVE / Pool ----
             last = (s == STEPS - 1)
             h32_new = p_h.tile([128, KT, SH], dt.float32r, tag="h32",
                                name="h32_new")
             for half, eng in ((0, nc.vector), (1, nc.gpsimd)):
                 hs = slice(2 * half, 2 * half + 2)
                 s1 = p_tmp.tile([128, 2, SH], dt.float32, tag=f"gsA{half}",
                                 name=f"gsA{half}")
                 eng.tensor_sub(s1[:], ht_big[:, hs, :], h32_prev[:, hs, :])
                 s2 = p_tmp.tile([128, 2, SH], dt.float32, tag=f"gsB{half}",
                                 name=f"gsB{half}")
                 eng.tensor_mul(s2[:], z_big[:, hs, :], s1[:])
                 eng.tensor_add(h32_new[:, hs, :], h32_prev[:, hs, :], s2[:])
             if DEBUG_DUMPS and s == 0 and rep == 0:
                 nc.scalar.dma_start(
                     dbg_hp_p[:].rearrange("(k p) n -> p k n", p=128),
                     h32_new[:])
             if last:
                 nc.sync.dma_start(out_p[:].rearrange("(k p) n -> p k n",
                                                      p=128), h32_new[:])
             else:
                 ag_in = dram.tile([D, SH], dt.float16, tag="ag_in",
                                   name="ag_in")
                 # cast-DMA fp32r -> fp16 straight into the AG input
                 nc.gpsimd.dma_start(
                     ag_in[:].rearrange("(k p) n -> p k n", p=128),
                     h32_new[:])
                 ag_out = dram.tile([NC_CORES * D, SH], dt.float16,
                                    tag="ag_out", name="ag_out",
                                    addr_space="Shared")
                 if "cc" in ablate or "ag" in ablate:
                     nc.sync.dma_start(ag_out[0:D, :], ag_in[:])
                 else:
                     nc.gpsimd.collective_compute(
                         "AllGather", mybir.AluOpType.bypass, replica_groups=RG,
                         ins=[ag_in[:]], outs=[ag_out[:]])
                 ag_out_prev = ag_out
                 h32_prev = h32_new

    nc.finalize()
    return nc


_BUILT = None
TRACE = False
LAST_RESULT = None
_BUILT_R = {}


def _get_built(repeats=1, ablate=()):
    global _BUILT
    key = (repeats, tuple(ablate))
    if key != (1, ()):
        if key not in _BUILT_R:
            _BUILT_R[key] = build(repeats, ablate)
        return _BUILT_R[key]
    if _BUILT is None:
        _BUILT = build()
    return _BUILT


def prepare_in_maps(adjacency, annotations, W_prop, b_prop, Wz, Uz, bz,
                    Wr, Ur, br, Wh, Uh, bh):
    A = np.asarray(adjacency, np.float32)
    ann = np.asarray(annotations, np.float32)
    W_prop = np.asarray(W_prop, np.float32)
    b_prop = np.asarray(b_prop, np.float32)
    gw_all = _q12(np.stack([np.asarray(x, np.float32)
                            for x in (Wz, Uz, Wr, Ur, Wh, Uh)]))
    bz = np.asarray(bz, np.float32).reshape(D, 1)
    br = np.asarray(br, np.float32).reshape(D, 1)
    bh = np.asarray(bh, np.float32).reshape(D, 1)

    h0 = np.zeros((N, D), np.float32)
    h0[:, :ann.shape[1]] = ann
    h0t = np.ascontiguousarray(h0.T)           # [D, N] fp32
    h0t_r = _q12(h0t)
    A_T = np.ascontiguousarray(A.T)            # [2E*N, N]

    # natural shard order: core c owns nodes 256c..256c+255
    h0ag = np.ascontiguousarray(
        h0t.reshape(D, NC_CORES, SH).transpose(1, 0, 2).reshape(
            NC_CORES * D, SH)).astype(np.float16)

    in_maps = []
    for c in range(NC_CORES):
        in_maps.append({
            "at": np.ascontiguousarray(
                A_T[c * N:(c + 1) * N, :]).astype(np.float16),
            "h0ag": h0ag,
            "h0sr": np.ascontiguousarray(h0t_r[:, c * SH:(c + 1) * SH]),
            "wc": _q12(W_prop[c]),
            "gw": gw_all,
            "bpc": np.ascontiguousarray(b_prop[c].reshape(1, D)),
            "bzc": bz, "brc": br, "bhc": bh,
        })
    return in_maps


def kernel(**inputs):
    from concourse.bass_utils import run_bass_kernel_spmd

    in_maps = prepare_in_maps(
        **{k: inputs[k] for k in ("adjacency", "annotations", "W_prop", "b_prop",
                                  "Wz", "Uz", "bz", "Wr", "Ur", "br",
                                  "Wh", "Uh", "bh")})
    nc = _get_built()
    res = run_bass_kernel_spmd(nc, in_maps, list(range(NC_CORES)), trace=TRACE)
    global LAST_RESULT
    LAST_RESULT = res
    h = np.empty((N, D), np.float32)
    for c in range(NC_CORES):
        h[c * SH:(c + 1) * SH] = res.results[c]["out"].T
    return h


# revision 36
# speedup vs baseline: 1.1255x; 1.0775x over previous
"""GGNN (gated graph NN) message-passing kernel for 8 Trainium2 NeuronCores.

Sharding: edge-type sharding. Core c owns edge-type block c of the adjacency
matrix (columns c*N..(c+1)*N of the [N, 2E*N] adjacency, pre-transposed on the
host) plus node shard c (nodes 256c..256c+255) for the GRU update.

Per step, on core c:
  U-pre:  pz/pr += Uz/Ur-terms on local h shard   (covers the AllGather wait)
  stage1: t_c = h @ W_c                            [N, D]   fp16 out
          (step 0 contracts only d<256: h0 columns 256.. are zero)
  stage2: aT partials, transposed orientation      [D, N]   fp16 x fp16
  RS:     4x quarter ReduceScatter (fp16), one per f-chunk; each quarter's
          output IS the aT k-chunk shard (cast-DMA'd to fp32r)
  GRU:    h_shard' = GRU(aT_shard, h_shard)        (fp32r matmuls)
  AG:     h^T' = AllGather(h_shard'^T, fp16)

Numerics: stage1+GRU matmuls float32r (full PE rate at free-dim>=256);
stage2 fp16 x fp16 (adjacency 0/1 exact in fp16); RS/AG wires fp16; fp32
accumulation in PSUM; h state kept in fp32r; elementwise tail fused into
wide [128, 512] ops split across DVE and Pool.
"""
import sys
if "/opt/trn_rl_repo" not in sys.path:
    sys.path.insert(0, "/opt/trn_rl_repo")

import numpy as np

NC_CORES = 8
N = 2048          # nodes
D = 512           # state dim
ANN = 256         # annotation dim
STEPS = 5
SH = N // NC_CORES   # 256 nodes per shard
KT = D // 128        # 4
MT = N // 128        # 16


def _q12(x):
    """Round fp32 to 12 explicit mantissa bits (fp32r grid), RNE."""
    mant, ex = np.frexp(np.asarray(x, np.float32).astype(np.float64))
    return (np.round(mant * 4096) / 4096 * np.exp2(ex)).astype(np.float32)


DEBUG_DUMPS = False
RS_WAYS = 4   # ReduceScatter splits per step: 4 (per f-chunk) or 2 (pairs)


def build(repeats=1, ablate=()):
    import concourse.bacc as bacc
    import concourse.mybir as mybir
    import concourse.tile as tile

    dt = mybir.dt
    nc = bacc.Bacc()
    at_p = nc.declare_dram_parameter("at", [N, N], dt.float16, isOutput=False)
    h0ag_p = nc.declare_dram_parameter("h0ag", [NC_CORES * D, SH], dt.float16,
                                       isOutput=False)
    h0sr_p = nc.declare_dram_parameter("h0sr", [D, SH], dt.float32r, isOutput=False)
    wc_p = nc.declare_dram_parameter("wc", [D, D], dt.float16, isOutput=False)
    gw_p = nc.declare_dram_parameter("gw", [6, D, D], dt.float32r, isOutput=False)
    bpc_p = nc.declare_dram_parameter("bpc", [1, D], dt.float32, isOutput=False)
    bz_p = nc.declare_dram_parameter("bzc", [D, 1], dt.float32, isOutput=False)
    br_p = nc.declare_dram_parameter("brc", [D, 1], dt.float32, isOutput=False)
    bh_p = nc.declare_dram_parameter("bhc", [D, 1], dt.float32, isOutput=False)
    out_p = nc.declare_dram_parameter("out", [D, SH], dt.float32r, isOutput=True)
    if DEBUG_DUMPS:
        dbg_hp_p = nc.declare_dram_parameter("dbg_hp", [D, SH], dt.float32r,
                                             isOutput=True)
        dbg_a32_p = nc.declare_dram_parameter("dbg_a32", [D, SH], dt.float32r,
                                              isOutput=True)
    RG = [list(range(NC_CORES))]
    Act = mybir.ActivationFunctionType

    from contextlib import ExitStack
    with tile.TileContext(nc) as tc, ExitStack() as stk:
        res = stk.enter_context(tc.tile_pool(name="res", bufs=1))
        # PSUM: p_u holds the 4 z/r gate accumulators across the step;
        # p_acc (4 banks) cycles through stage1 pt / stage2 paT / ht.
        p_u = stk.enter_context(tc.tile_pool(name="pu", bufs=1, space="PSUM"))
        p_acc = stk.enter_context(tc.tile_pool(name="pacc", bufs=4, space="PSUM"))
        p_hc16 = stk.enter_context(tc.tile_pool(name="phc16", bufs=2))
        p_t = stk.enter_context(tc.tile_pool(name="pt", bufs=1))
        p_asb = stk.enter_context(tc.tile_pool(name="pasb", bufs=4))
        p_sm = stk.enter_context(tc.tile_pool(name="psm", bufs=1))
        p_tmp = stk.enter_context(tc.tile_pool(name="ptmp", bufs=1))
        p_h = stk.enter_context(tc.tile_pool(name="ph", bufs=2))
        dram = stk.enter_context(tc.tile_pool(name="dram", bufs=2, space="DRAM"))

        # ---- setup: constants, weights, adjacency ----
        ones = res.tile([1, 128], dt.float32, tag="ones")
        nc.vector.memset(ones[:], 1.0)
        bpc_t = res.tile([1, D], dt.float32, tag="bpc")
        nc.sync.dma_start(bpc_t[:], bpc_p[:])
        pb = p_acc.tile([128, D], dt.float32, tag="mm")
        nc.tensor.matmul(pb[:], ones[:], bpc_t[:], start=True, stop=True)
        bias_bcast = res.tile([128, D], dt.float32, tag="bias_bcast")
        nc.vector.tensor_copy(bias_bcast[:], pb[:])

        bias_tiles = {}
        for nm, par in (("z", bz_p), ("r", br_p), ("h", bh_p)):
            for f in range(KT):
                bt = res.tile([128, 1], dt.float32, tag=f"b{nm}{f}")
                nc.sync.dma_start(bt[:], par[f * 128:(f + 1) * 128, :])
                bias_tiles[(nm, f)] = bt

        wc_t = []
        for k in range(KT):
            w = res.tile([128, D], dt.float16, tag=f"wc{k}")
            nc.sync.dma_start(w[:], wc_p[k * 128:(k + 1) * 128, :])
            wc_t.append(w)

        at_t = []
        for m in range(MT):
            a = res.tile([128, N], dt.float16, tag=f"at{m}")
            nc.sync.dma_start(a[:], at_p[m * 128:(m + 1) * 128, :])
            at_t.append(a)

        gw_res = []
        for g in range(6):
            w = res.tile([128, KT, D], dt.float32r, tag=f"gwr{g}")
            nc.scalar.dma_start(w[:], gw_p[g].rearrange("(k p) f -> p k f", p=128))
            gw_res.append(w)

        for rep in range(repeats):
          # h state: one fp32r big tile [128, KT, SH]; slice [:, k, :] is the
          # matmul rhs for k-chunk k.
          h32_prev = p_h.tile([128, KT, SH], dt.float32r, tag="h32",
                              name="h32_init")
          nc.sync.dma_start(h32_prev[:],
                            h0sr_p[:].rearrange("(k p) n -> p k n", p=128))

          ag_out_prev = None

          for s in range(STEPS):
             KT_s = 2 if s == 0 else KT   # h0 cols 256.. are zero
             # ---- z/r U-terms (local h only; cover the AG wait) ----
             zr_ps = {}
             for g in ("z", "r"):
                 for fp in range(2):
                     zr_ps[(g, fp)] = p_u.tile([128, 2 * SH], dt.float32,
                                               tag=f"u{g}{fp}",
                                               name=f"u{g}{fp}")
             if "gru" not in ablate:
                 for g, uidx in (("z", 1), ("r", 3)):
                     Uq = gw_res[uidx]
                     for fp in range(2):
                         for j in range(2):
                             f = 2 * fp + j
                             for k in range(KT):
                                 nc.tensor.matmul(
                                     zr_ps[(g, fp)][:, j * SH:(j + 1) * SH],
                                     Uq[:, k, f * 128:(f + 1) * 128],
                                     h32_prev[:, k, :],
                                     start=(j == 0 and k == 0), stop=False)

             # ---- stage 1: t = h @ W_c (+ b_c); t stored fp16 ----
             # hT arrives via 4 cast-DMAs (fp16 -> fp32r) of 2 shards each.
             hc32 = []
             if "s1" not in ablate:
                 src = (h0ag_p if s == 0 else ag_out_prev)
                 view = src[:].rearrange("(c k p) n -> p k c n", p=128, k=KT)
                 for cp in range(4):
                     hc16 = p_hc16.tile([128, KT, 2, SH], dt.float16,
                                        tag="hc16")
                     for ci in range(2):
                         nc.sync.dma_start(hc16[:, :, ci, :],
                                           view[:, :, 2 * cp + ci, :])
                     hc32.append(hc16)
             t_tiles = [None] * MT
             for m in range(MT):
                 pt = p_acc.tile([128, D], dt.float32, tag="mm")
                 if "s1" in ablate:
                     nc.tensor.matmul(pt[:], wc_t[0][:, 0:128], wc_t[1][:],
                                      start=True, stop=True)
                 else:
                     hc = hc32[m // 4]
                     ci, half = (m // 2) % 2, m % 2
                     for k in range(KT_s):
                         nc.tensor.matmul(
                             pt[:], hc[:, k, ci, half * 128:half * 128 + 128],
                             wc_t[k][:], start=(k == 0), stop=(k == KT_s - 1))
                 tm = p_t.tile([128, D], dt.float16, tag=f"t{m}")
                 nc.vector.tensor_add(tm[:], pt[:], bias_bcast[:])
                 t_tiles[m] = tm

             # ---- stage 2 + split RS: aT in transposed orientation ----
             # f-chunk fc covers f-rows fc*128..fc*128+127; each RS group's
             # output is the corresponding aT k-chunk shard(s).
             FPG = KT // RS_WAYS    # f-chunks per RS group
             aT = [None] * KT
             rs_in = None
             for fc in range(KT):
                 gi = fc % FPG
                 if gi == 0:
                     rs_in = dram.tile([FPG * N // 2, SH], dt.float16,
                                       tag=f"rs_in{fc // FPG}",
                                       name=f"rs_in{fc // FPG}")
                 asb_cnt = 0
                 for dhalf in range(2):
                     pas = [p_acc.tile([128, D], dt.float32, tag="mm",
                                       name=f"pa{fc}_{dhalf}_{j}")
                            for j in range(2)]
                     if "s2" in ablate:
                         for j in range(2):
                             nc.tensor.matmul(pas[j][:], t_tiles[0][:, 0:128],
                                              t_tiles[1][:], start=True,
                                              stop=True)
                     else:
                         for k in range(MT):
                             lhs = t_tiles[k][:, fc * 128:(fc + 1) * 128]
                             for j in range(2):
                                 nc.tensor.matmul(
                                     pas[j][:], lhs,
                                     at_t[k][:, dhalf * 1024 + j * 512:
                                             dhalf * 1024 + (j + 1) * 512],
                                     start=(k == 0), stop=(k == MT - 1))
                     for j in range(2):
                         asb = p_asb.tile([128, D], dt.float16, tag="asb")
                         if asb_cnt % 2 == 0:
                             nc.scalar.copy(asb[:], pas[j][:])
                         else:
                             nc.vector.tensor_copy(asb[:], pas[j][:])
                         # dest rows: shard-pair sp covers shards 2sp, 2sp+1
                         sp = dhalf * 2 + j
                         dst = rs_in[:].rearrange(
                             "(q s2 g p) n2 -> p q s2 g n2",
                             q=4, s2=2, g=FPG, p=128)
                         eng = nc.sync if asb_cnt % 2 == 0 else nc.scalar
                         if FPG == 1:
                             dsl = dst[:, sp, :, 0, :]
                         else:
                             dsl = dst[:, sp, :, gi, :]
                         eng.dma_start(
                             dsl,
                             asb[:].rearrange("p (s2 n2) -> p s2 n2", s2=2))
                         asb_cnt += 1
                 if gi == FPG - 1:
                     grp = fc // FPG
                     rs_out = dram.tile([FPG * 128, SH], dt.float16,
                                        tag=f"rs_out{grp}",
                                        name=f"rs_out{grp}")
                     if "cc" in ablate or "rs" in ablate:
                         nc.sync.dma_start(rs_out[:], rs_in[0:FPG * 128, :])
                     else:
                         nc.gpsimd.collective_compute(
                             "ReduceScatter", mybir.AluOpType.add,
                             replica_groups=RG,
                             ins=[rs_in[:]], outs=[rs_out[:]])
                     # rs_out fp16 == aT chunk shard(s); cast-DMA to fp32r
                     for g2 in range(FPG):
                         kk = grp * FPG + g2
                         a_k = p_sm.tile([128, SH], dt.float32r, tag=f"aT{kk}",
                                         name=f"aT{kk}")
                         nc.gpsimd.dma_start(
                             a_k[:], rs_out[g2 * 128:(g2 + 1) * 128, :])
                         aT[kk] = a_k
                         if DEBUG_DUMPS and s == 1 and rep == 0:
                             nc.scalar.dma_start(
                                 dbg_a32_p[kk * 128:(kk + 1) * 128, :], a_k[:])

             # ---- GRU gates ----
             if "gru" in ablate:
                 for g in ("z", "r"):
                     for fp in range(2):
                         for j in range(2):
                             nc.tensor.matmul(
                                 zr_ps[(g, fp)][:, j * SH:(j + 1) * SH],
                                 aT[0][:, 0:128], aT[0][:],
                                 start=False, stop=(j == 1))
             else:
                 # W-terms k-major: k-chunk k runs as soon as quarter-RS k
                 # has landed (while later quarters are still in flight).
                 for k in range(KT):
                     for g, widx in (("z", 0), ("r", 2)):
                         Wq = gw_res[widx]
                         for fp in range(2):
                             for j in range(2):
                                 f = 2 * fp + j
                                 nc.tensor.matmul(
                                     zr_ps[(g, fp)][:, j * SH:(j + 1) * SH],
                                     Wq[:, k, f * 128:(f + 1) * 128],
                                     aT[k][:],
                                     start=False,
                                     stop=(k == KT - 1 and j == 1))
             z_big = p_sm.tile([128, KT, SH], dt.float32, tag="z_big")
             r_big = p_sm.tile([128, KT, SH], dt.float32, tag="r_big")
             for g, big in (("z", z_big), ("r", r_big)):
                 for f in range(KT):
                     nc.scalar.activation(
                         big[:, f, :], zr_ps[(g, f // 2)][:, (f % 2) * SH:
                                                          (f % 2 + 1) * SH],
                         Act.Sigmoid, bias=bias_tiles[(g, f)][:])
             rh_big = p_sm.tile([128, KT, SH], dt.float32r, tag="rh_big")
             nc.vector.tensor_mul(rh_big[:], r_big[:], h32_prev[:])
             # h-tilde gate: U-term on rh, then W-term on aT
             ht_ps = [p_acc.tile([128, 2 * SH], dt.float32, tag="mm",
                                 name=f"ht{fp}") for fp in range(2)]
             if "gru" in ablate:
                 for fp in range(2):
                     nc.tensor.matmul(ht_ps[fp][:, 0:SH], rh_big[:, 0, 0:128],
                                      rh_big[:, 0, :], start=True, stop=True)
                     nc.tensor.matmul(ht_ps[fp][:, SH:2 * SH],
                                      rh_big[:, 0, 0:128],
                                      rh_big[:, 0, :], start=True, stop=True)
             else:
                 # W-terms (aT) first: they don't depend on r/rh, so they run
                 # while the sigmoid/rh chain is still in flight.
                 for fp in range(2):
                     for j in range(2):
                         f = 2 * fp + j
                         for k in range(KT):
                             nc.tensor.matmul(
                                 ht_ps[fp][:, j * SH:(j + 1) * SH],
                                 gw_res[4][:, k, f * 128:(f + 1) * 128],
                                 aT[k][:],
                                 start=(j == 0 and k == 0), stop=False)
                 for fp in range(2):
                     for j in range(2):
                         f = 2 * fp + j
                         for k in range(KT):
                             nc.tensor.matmul(
                                 ht_ps[fp][:, j * SH:(j + 1) * SH],
                                 gw_res[5][:, k, f * 128:(f + 1) * 128],
                                 rh_big[:, k, :], start=False,
                                 stop=(k == KT - 1 and j == 1))
             ht_big = p_sm.tile([128, KT, SH], dt.float32, tag="ht_big")
             for f in range(KT):
                 nc.scalar.activation(
                     ht_big[:, f, :], ht_ps[f // 2][:, (f % 2) * SH:
                                                    (f % 2 + 1) * SH],
                     Act.Tanh, bias=bias_tiles[("h", f)][:])

             # ---- h' = h + z * (ht - h), wide ops split DVE / Pool ----
             last = (s == STEPS - 1)
             h32_new = p_h.tile([128, KT, SH], dt.float32r, tag="h32",
                                name="h32_new")
             for half, eng in ((0, nc.vector), (1, nc.gpsimd)):
                 hs = slice(2 * half, 2 * half + 2)
                 s1 = p_tmp.tile([128, 2, SH], dt.float32, tag=f"gsA{half}",
                                 name=f"gsA{half}")
                 eng.tensor_sub(s1[:], ht_big[:, hs, :], h32_prev[:, hs, :])
                 s2 = p_tmp.tile([128, 2, SH], dt.float32, tag=f"gsB{half}",
                                 name=f"gsB{half}")
                 eng.tensor_mul(s2[:], z_big[:, hs, :], s1[:])
                 eng.tensor_add(h32_new[:, hs, :], h32_prev[:, hs, :], s2[:])
             if DEBUG_DUMPS and s == 0 and rep == 0:
                 nc.scalar.dma_start(
                     dbg_hp_p[:].rearrange("(k p) n -> p k n", p=128),
                     h32_new[:])
             if last:
                 nc.sync.dma_start(out_p[:].rearrange("(k p) n -> p k n",
                                                      p=128), h32_new[:])
             else:
                 ag_in = dram.tile([D, SH], dt.float16, tag="ag_in",
                                   name="ag_in")
                 # cast-DMA fp32r -> fp16 straight into the AG input
                 nc.gpsimd.dma_start(
                     ag_in[:].rearrange("(k p) n -> p k n", p=128),
                     h32_new[:])
                 ag_out = dram.tile([NC_CORES * D, SH], dt.float16,
                                    tag="ag_out", name="ag_out",
                                    addr_space="Shared")
                 if "cc" in ablate or "ag" in ablate:
                     nc.sync.dma_start(ag_out[0:D, :], ag_in[:])
                 else:
                     nc.gpsimd.collective_compute(
                         "AllGather", mybir.AluOpType.bypass, replica_groups=RG,
                         ins=[ag_in[:]], outs=[ag_out[:]])
                 ag_out_prev = ag_out
                 h32_prev = h32_new

    nc.finalize()
    return nc


_BUILT = None
TRACE = False
LAST_RESULT = None
_BUILT_R = {}


def _get_built(repeats=1, ablate=()):
    global _BUILT
    key = (repeats, tuple(ablate))
    if key != (1, ()):
        if key not in _BUILT_R:
            _BUILT_R[key] = build(repeats, ablate)
        return _BUILT_R[key]
    if _BUILT is None:
        _BUILT = build()
    return _BUILT


def prepare_in_maps(adjacency, annotations, W_prop, b_prop, Wz, Uz, bz,
                    Wr, Ur, br, Wh, Uh, bh):
    A = np.asarray(adjacency, np.float32)
    ann = np.asarray(annotations, np.float32)
    W_prop = np.asarray(W_prop, np.float32)
    b_prop = np.asarray(b_prop, np.float32)
    gw_all = _q12(np.stack([np.asarray(x, np.float32)
                            for x in (Wz, Uz, Wr, Ur, Wh, Uh)]))
    bz = np.asarray(bz, np.float32).reshape(D, 1)
    br = np.asarray(br, np.float32).reshape(D, 1)
    bh = np.asarray(bh, np.float32).reshape(D, 1)

    h0 = np.zeros((N, D), np.float32)
    h0[:, :ann.shape[1]] = ann
    h0t = np.ascontiguousarray(h0.T)           # [D, N] fp32
    h0t_r = _q12(h0t)
    A_T = np.ascontiguousarray(A.T)            # [2E*N, N]

    # natural shard order: core c owns nodes 256c..256c+255
    h0ag = np.ascontiguousarray(
        h0t.reshape(D, NC_CORES, SH).transpose(1, 0, 2).reshape(
            NC_CORES * D, SH)).astype(np.float16)

    in_maps = []
    for c in range(NC_CORES):
        in_maps.append({
            "at": np.ascontiguousarray(
                A_T[c * N:(c + 1) * N, :]).astype(np.float16),
            "h0ag": h0ag,
            "h0sr": np.ascontiguousarray(h0t_r[:, c * SH:(c + 1) * SH]),
            "wc": W_prop[c].astype(np.float16),
            "gw": gw_all,
            "bpc": np.ascontiguousarray(b_prop[c].reshape(1, D)),
            "bzc": bz, "brc": br, "bhc": bh,
        })
    return in_maps


def kernel(**inputs):
    from concourse.bass_utils import run_bass_kernel_spmd

    in_maps = prepare_in_maps(
        **{k: inputs[k] for k in ("adjacency", "annotations", "W_prop", "b_prop",
                                  "Wz", "Uz", "bz", "Wr", "Ur", "br",
                                  "Wh", "Uh", "bh")})
    nc = _get_built()
    res = run_bass_kernel_spmd(nc, in_maps, list(range(NC_CORES)), trace=TRACE)
    global LAST_RESULT
    LAST_RESULT = res
    h = np.empty((N, D), np.float32)
    for c in range(NC_CORES):
        h[c * SH:(c + 1) * SH] = res.results[c]["out"].T
    return h
